# revision 16
# baseline (speedup 1.0000x reference)
"""CARAFE3D Trainium2 kernel (8 NeuronCores, SPMD, host scatter/gather).

Math (reference): for x [N=2, C1=32, H=W=D=32]:
  t   = conv1x1(x, w_down) + b_down                  [N, 8, 32,32,32]
  enc = conv3x3x3(t, w_enc, pad=1) + b_enc           [N, 216, ...]
  kern= softmax over k3=27 of enc.reshape(27, 8, ...)
  out[c,p,r] = sum_k patches(x)[c,p,k] * kern[k,r,p]
  y   = pixel_shuffle(out, R=2) -> conv1x1(w_out) + b_out   [N, 16, 64,64,64]

Kernel trick: fold w_out into a pre-pass y1 = w_out @ x (channels 32->16 on the
coarse grid; valid since the final conv is 1x1 and softmax weights sum to 1),
so the reassembly contracts [27] per voxel with only 16 channels and the fine
grid 1x1 conv disappears. b_out is added on the host at the end.

Sharding: 8 cores = (n in 0..2) x (h-block in 0..4); each core gets a
zero-padded slice xp [32, 10, 34, 34] (halo 1) and computes its own
[16, 16, 64, 64] fine-output block. No collectives.

Per-core device pipeline (all f32):
  stage1: one matmul pass xp -> cat = [t8 (8ch) ; y1 (16ch)]  [24, 11560]
  t_dw:   9 shifted partition-placement DMA copies -> [73, 11560] (+ones row)
  per 128-voxel chunk (4 w-columns x 32 d):
    enc:  3 accumulated matmuls (kh taps) -> PSUM [128v, 216ch]
          stationary = t_dw slice, moving = W_enc_k [73, 216] (ch = r*27+k)
    exp:  ACT Exp PSUM->SBUF (b_enc folded into ones-row of W_enc_1)
    s:    DVE reduce over k (free dim) -> [128, 8]; reciprocal
    norm: DVE mult with step-0 broadcast, output AP strided (k + 32*r)
    T3:   DVE 32x32 stream transpose -> EnT[32*b + k, 32*r + d] per w-col b
  PATCH:  im2col of y1 via 108 DMAs/slab -> [32*b + k, o*256 + 32*c + d]
  reassembly: per voxel one matmul on a 32x32 PE tile (b = w%4):
          lhsT = EnT[32b:32b+27, r-strided 8], rhs = PATCH[32b:32b+27, o-strided 16]
          out PSUM[32b+r, d*16+o]; one PSUM bank per chunk -> DMA to DRAM.
Host: unshuffle [s, c, b, r, d, o] -> [o, H, W, D] and add b_out.
"""

import numpy as np

C1, C2, CM, K, R = 32, 16, 8, 3, 2
N, H, W, D = 2, 32, 32, 32
K3, R3 = 27, 8
HB = 4            # h-blocks per batch element
HS = H // HB      # 8 h planes per core
NCORES = 8
HP, WP, DP = HS + 2, W + 2, D + 2      # 10, 34, 34
FLAT = HP * WP * DP                    # 11560
SH = WP * DP                           # 1156  (h stride)
NCHUNK = 8                             # w-quads per slab
NVOX_CHUNK = 128                       # 4 w * 32 d
CHUNKS = HS * NCHUNK                   # 64 chunks/core


# ---------------------------------------------------------------- host prep

def _prep_weights(w_down, b_down, w_enc, b_enc, w_out):
    """Host-side weight layouts."""
    w_down = np.asarray(w_down, np.float32).reshape(CM, C1)
    w_out = np.asarray(w_out, np.float32).reshape(C2, C1)
    w_enc = np.asarray(w_enc, np.float32).reshape(216, CM, K, K, K)
    b_enc = np.asarray(b_enc, np.float32).reshape(216)
    b_down = np.asarray(b_down, np.float32).reshape(CM)

    wcat = np.zeros((C1, 24), np.float32)
    wcat[:, 0:8] = w_down.T
    wcat[:, 8:24] = w_out.T
    bcat = np.zeros((24, 1), np.float32)
    bcat[0:8, 0] = b_down

    # enc channel remap ck=(k*8+r) -> ch'=(r*27+k)
    ck_of_ch = (np.arange(216) % 27) * 8 + (np.arange(216) // 27)
    # W_enc_mats[i, 8*(3j+l)+c, ch'] = w_enc[ck(ch'), c, i, j, l]
    t = w_enc.transpose(2, 3, 4, 1, 0)          # [i, j, l, c, ck]
    t = t[..., ck_of_ch]                        # [i, j, l, c, ch']
    wenc = np.zeros((K, 73, 216), np.float32)
    wenc[:, :72, :] = t.reshape(K, 9 * CM, 216).astype(np.float32)
    wenc[1, 72, :] = b_enc[ck_of_ch]            # bias via ones row, center tap
    return wcat, bcat, wenc


def _prep_x(x):
    """Slice + zero-pad per core: xp [8][32, FLAT]."""
    x = np.asarray(x, np.float32)
    xpad = np.zeros((N, C1, HP + (HB - 1) * HS, WP, DP), np.float32)
    xpad[:, :, 1:H + 1, 1:W + 1, 1:D + 1] = x
    cores = []
    for core in range(NCORES):
        n, hb = core // HB, core % HB
        sl = xpad[n, :, hb * HS:hb * HS + HP, :, :]
        cores.append(np.ascontiguousarray(sl.reshape(C1, FLAT)))
    return cores


def _unshuffle(raws, b_out):
    """raws: 8 arrays [HS, NCHUNK, 4, 8, 32, 16] -> full [N, C2, 64, 64, 64]."""
    out = np.empty((N, C2, H * R, W * R, D * R), np.float32)
    for core in range(NCORES):
        n, hb = core // HB, core % HB
        a = raws[core].reshape(HS, NCHUNK, 4, 2, 2, 2, D, C2)
        # [s, c, b, rh, rw, rd, d, o] -> [o, s, rh, c, b, rw, d, rd]
        y = a.transpose(7, 0, 3, 1, 2, 4, 6, 5).reshape(C2, HS * R, W * R, D * R)
        out[n, :, hb * HS * R:(hb + 1) * HS * R] = y
    out += np.asarray(b_out, np.float32).reshape(1, C2, 1, 1, 1)
    return out


# ------------------------------------------------------------ emulation path

def _emulate_core(xp, wcat, bcat, wenc):
    """Numpy mirror of the device program for one core. Returns raw out_dev."""
    cat = wcat.T @ xp + bcat                     # [24, FLAT]
    # border zeroing: only w/d borders (always global padding); the h planes
    # 0 and 9 are real halo data from neighboring h-blocks (zero-padded on
    # the host already for edge cores).
    g = cat.reshape(24, HP, WP, DP)
    g[:, :, 0, :] = 0.0
    g[:, :, WP - 1, :] = 0.0
    g[:, :, :, 0] = 0.0
    g[:, :, :, DP - 1] = 0.0
    cat = g.reshape(24, FLAT)
    t8, y1 = cat[0:8], cat[8:24]

    # t_dw [73, FLAT]
    t_dw = np.zeros((73, FLAT), np.float32)
    t_dw[72] = 1.0
    for j in range(K):
        for l in range(K):
            off = (j - 1) * DP + (l - 1)
            s0, s1 = max(0, off), FLAT + min(0, off)
            d0 = max(0, -off)
            t_dw[8 * (3 * j + l):8 * (3 * j + l) + 8, d0:d0 + (s1 - s0)] = \
                t8[:, s0:s1]

    out_dev = np.zeros((HS, NCHUNK, 4, R3, D, C2), np.float32)
    for s in range(HS):
        h = s + 1
        # PATCH [128, 16*256] per slab
        patch = np.zeros((128, C2 * 256), np.float32)
        qs = np.arange(NCHUNK) * 4 * DP
        ds = np.arange(D)
        for b in range(4):
            for kf in range(K3):
                i, j, l = kf // 9, (kf // 3) % 3, kf % 3
                base = (h + i - 1) * SH + (b + j) * DP + l
                idx = base + qs[:, None] + ds[None, :]           # [8, 32]
                patch[32 * b + kf] = y1[:, idx].reshape(C2 * 256)
        for c in range(NCHUNK):
            vbase = h * SH + (1 + 4 * c) * DP + 1
            cols = (vbase + np.arange(4)[:, None] * DP
                    + np.arange(D)[None, :]).reshape(-1)    # 128 voxel cols
            psum = np.zeros((128, 216), np.float32)
            for kh in range(K):
                lhsT = t_dw[:, cols + (kh - 1) * SH]         # [73, 128]
                psum += lhsT.T @ wenc[kh]                    # [128, 216]
            en = np.exp(psum)                                # [128, (r*27+k)]
            sr = en.reshape(128, 8, 27).sum(-1)              # [128, 8]
            enn = np.zeros((128, 256), np.float32)
            enn.reshape(128, 8, 32)[:, :, :27] = \
                en.reshape(128, 8, 27) / sr[:, :, None]
            # stream transpose 32x32 blocks
            ent = enn.reshape(4, 32, 8, 32).transpose(0, 3, 2, 1) \
                     .reshape(128, 256)                      # [32b+k, 32r+d]
            E = ent.reshape(4, 32, 8, 32)[:, :27]            # [b, k, r, d]
            P = patch.reshape(4, 32, C2, 8, 32)[:, :27, :, c, :]   # [b,k,o,d]
            out_dev[s, c] = np.einsum('bkrd,bkod->brdo', E, P)
    return out_dev


# -------------------------------------------------------------- device path

_DEVICE_CACHE = {}


def _mkap(tileobj, part0, nparts, base, dims):
    """Raw AP on a pool tile: partition range [part0, part0+nparts), free
    offset `base`, free dims = [[step, count], ...]."""
    import concourse.bass as bass
    full = tileobj[:]
    pitch = full.ap[0][0]
    return bass.AP(full.tensor, full.offset + part0 * pitch + base,
                   [[pitch, nparts]] + [list(d) for d in dims])


def _build_program():
    import concourse.bacc as bacc
    import concourse.mybir as mybir
    import concourse.tile as tile

    f32 = mybir.dt.float32
    AF = mybir.ActivationFunctionType
    nc = bacc.Bacc("TRN2", target_bir_lowering=False, debug=False,
                   num_devices=NCORES)
    xp_d = nc.dram_tensor("xp", [C1, FLAT], f32, kind="ExternalInput")
    wcat_d = nc.dram_tensor("wcat", [C1, 24], f32, kind="ExternalInput")
    bcat_d = nc.dram_tensor("bcat", [24, 1], f32, kind="ExternalInput")
    wenc_d = nc.dram_tensor("wenc", [K, 73, 216], f32, kind="ExternalInput")
    out_d = nc.dram_tensor("out", [HS, NCHUNK, 4, R3, D, C2], f32,
                           kind="ExternalOutput")

    TDW = 3 * 1024  # per-slab t_dw: 3 planes x 1024 interior voxel columns
    with tile.TileContext(nc) as tc:
        with (
            tc.tile_pool(name="persist", bufs=1) as persist,
            tc.tile_pool(name="xps", bufs=3) as xp_pool,
            tc.tile_pool(name="tdwp", bufs=2) as tdw_pool,
            tc.tile_pool(name="entp", bufs=2) as ent_pool,
            tc.tile_pool(name="chunk", bufs=4) as chunk_pool,
            tc.tile_pool(name="patchp", bufs=2) as patch_pool,
            tc.tile_pool(name="ps1", bufs=2, space="PSUM") as ps1,
            tc.tile_pool(name="psenc", bufs=2, space="PSUM") as psenc,
            tc.tile_pool(name="psout", bufs=3, space="PSUM") as psout,
        ):
            wcat = persist.tile([C1, 24], f32)
            nc.sync.dma_start(wcat[:], wcat_d[:])
            bcat = persist.tile([24, 1], f32)
            nc.sync.dma_start(bcat[:], bcat_d[:])
            wenc = persist.tile([73, K * 216], f32)
            for kh in range(K):
                nc.sync.dma_start(wenc[:, kh * 216:(kh + 1) * 216], wenc_d[kh])

            ones_row = persist.tile([1, TDW], f32)
            nc.vector.memset(ones_row[:], 1.0)

            cat = persist.tile([24, FLAT], f32)
            # stage 1: cat = wcat.T @ xp + bcat  (xp streamed from DRAM)
            nco = 0
            while nco < FLAT:
                n = min(512, FLAT - nco)
                xpc = xp_pool.tile([C1, 512], f32, tag="xpc")
                nc.sync.dma_start(xpc[:, 0:n], xp_d[:, nco:nco + n])
                acc1 = ps1.tile([24, 512], f32)
                nc.tensor.matmul(acc1[:, 0:n], wcat[:], xpc[:, 0:n])
                nc.scalar.activation(cat[:, nco:nco + n], acc1[:, 0:n],
                                     AF.Identity, bias=bcat[:])
                nco += n
            # border zeroing on cat: w/d borders only (h planes 0/9 are halo)
            catg = cat[:].rearrange("p (h w d) -> p h w d", h=HP, w=WP, d=DP)
            nc.vector.memset(catg[:, :, 0:WP:WP - 1, :], 0.0)
            nc.vector.memset(catg[:, :, :, 0:DP:DP - 1], 0.0)

            # ---- slab loop ----
            for s in range(HS):
                h = s + 1
                # per-slab t_dw, gathered to interior voxel columns:
                # t_dw[8*(3j+l)+c, kh*1024 + w*32 + d]
                #   = cat[c, (h-1+kh)*SH + (w+j)*DP + (d+l)]
                t_dw = tdw_pool.tile([73, TDW], f32)
                # ones row 72 via DMA (compute-engine APs need 32-aligned
                # base partitions; DMAs don't)
                nc.sync.dma_start(t_dw[72:73, :], ones_row[:])
                for j in range(K):
                    for l in range(K):
                        r0 = 8 * (3 * j + l)
                        for kh in range(K):
                            src = _mkap(cat, 0, 8,
                                        (h - 1 + kh) * SH + j * DP + l,
                                        [[DP, W], [1, D]])
                            nc.sync.dma_start(
                                t_dw[r0:r0 + 8,
                                     kh * 1024:(kh + 1) * 1024], src)
                ent = ent_pool.tile([128, 256 * NCHUNK], f32)
                patch = patch_pool.tile([128, C2 * 256], f32)
                for b in range(4):
                    for kf in range(K3):
                        i, j, l = kf // 9, (kf // 3) % 3, kf % 3
                        base = (h + i - 1) * SH + (b + j) * DP + l
                        dst = _mkap(patch, 32 * b + kf, 1, 0,
                                    [[256, C2], [32, NCHUNK], [1, D]])
                        src = _mkap(cat, 8, C2, base,
                                    [[4 * DP, NCHUNK], [1, D]])
                        nc.sync.dma_start(dst, src)

                for c in range(NCHUNK):
                    # enc + softmax for this chunk
                    acc = psenc.tile([128, 216], f32)
                    for kh in range(K):
                        st = t_dw[:, kh * 1024 + 128 * c:
                                  kh * 1024 + 128 * (c + 1)]
                        nc.tensor.matmul(acc[:], st,
                                         wenc[:, kh * 216:(kh + 1) * 216],
                                         start=(kh == 0), stop=(kh == 2))
                    en = chunk_pool.tile([128, 216], f32, tag="en")
                    nc.scalar.activation(en[:], acc[:], AF.Exp)
                    env = en[:].rearrange("p (r k) -> p r k", r=8, k=27)
                    sr = chunk_pool.tile([128, 8], f32, tag="sr")
                    nc.vector.tensor_reduce(sr[:], env,
                                            axis=mybir.AxisListType.X,
                                            op=mybir.AluOpType.add)
                    inv = chunk_pool.tile([128, 8], f32, tag="inv")
                    nc.vector.reciprocal(inv[:], sr[:])
                    enn = chunk_pool.tile([128, 256], f32, tag="enn")
                    nc.vector.memset(enn[:], 0.0)
                    ennv = _mkap(enn, 0, 128, 0, [[32, 8], [1, 27]])
                    invb = _mkap(inv, 0, 128, 0, [[1, 8], [0, 27]])
                    nc.vector.tensor_tensor(ennv, env, invb,
                                            op=mybir.AluOpType.mult)
                    nc.vector.transpose(ent[:, 256 * c:256 * (c + 1)], enn[:])

                    # reassembly for this chunk
                    pso = psout.tile([128, 512], f32)
                    for b in range(4):
                        for d in range(D):
                            lhs = _mkap(ent, 32 * b, 27, 256 * c + d,
                                        [[32, 8]])
                            rhs = _mkap(patch, 32 * b, 27, 32 * c + d,
                                        [[256, C2]])
                            nc.tensor.matmul(
                                pso[32 * b:32 * b + 8, 16 * d:16 * d + 16],
                                lhs, rhs, tile_position=(32 * b, 32 * b))
                    osb = chunk_pool.tile([128, 512], f32, tag="osb")
                    nc.scalar.activation(osb[:], pso[:], AF.Copy)
                    for b in range(4):
                        nc.sync.dma_start(out_d[s, c, b],
                                          osb[32 * b:32 * b + 8, :])
    nc.compile()
    return nc


def kernel(x, w_down, b_down, w_enc, b_enc, w_out, b_out, emulate=False):
    wcat, bcat, wenc = _prep_weights(w_down, b_down, w_enc, b_enc, w_out)
    xps = _prep_x(x)
    if emulate:
        raws = [_emulate_core(xps[c], wcat, bcat, wenc) for c in range(NCORES)]
        return _unshuffle(raws, b_out)

    import sys
    if '/opt/trn_rl_repo' not in sys.path:
        sys.path.insert(0, '/opt/trn_rl_repo')
    from concourse.bass_utils import run_bass_kernel_spmd
    if 'nc' not in _DEVICE_CACHE:
        _DEVICE_CACHE['nc'] = _build_program()
    nc = _DEVICE_CACHE['nc']
    in_maps = [{"xp": xps[c], "wcat": wcat, "bcat": bcat, "wenc": wenc}
               for c in range(NCORES)]
    res = run_bass_kernel_spmd(nc, in_maps, core_ids=list(range(NCORES)))
    raws = [res.results[c]["out"] for c in range(NCORES)]
    return _unshuffle(raws, b_out)
